# revision 11
# baseline (speedup 1.0000x reference)
"""Trainium2 Bass kernel for CayleyStringPE (RoPE + Cayley orthogonal mix).

Math: out = C @ rope(x) per token, where C = (I-S)(I+S)^{-1} is a fixed
128x128 orthogonal matrix (Cayley transform of the skew-symmetric S built
from s_params), and rope applies interleaved-pair rotations by angle
pos[t]*freqs[i].

Device formulation: rope(x)_t = x_t*c_t + P x_t * s_t with P the fixed
pair-swap-sign matrix and c_t/s_t the duplicated cos/sin vectors, so

    out_t = A @ (x_t * c_t) + Bm @ (x_t * s_t),   A = C,  Bm = C @ P

i.e. two 128x128 matmuls per token tile plus two elementwise multiplies.
No cross-partition shuffles on device.

Sharding: sequence-parallel across 8 cores (positions split 8 x 1024, all
batches on every core). cos/sin tables are per-core (128 x 1024) and reused
across the 8 batches. A/Bm replicated. No collectives.

Layout: tokens on the SBUF free axis, D=128 on partitions. Host pre-
transposes shards to (128, B*1024) D-major so all DMAs are contiguous.
"""

import sys

import numpy as np

for _p in ("/opt/trn_rl_repo", "/opt/pypackages"):
    if _p not in sys.path:
        sys.path.insert(0, _p)

B, N, D = 8, 8192, 128
NCORES = 8
NSH = N // NCORES          # positions per core
TOK = B * NSH              # tokens per core
CHUNK = 2048               # tokens per DMA chunk (1 MiB)
MMN = 512                  # matmul moving free dim (one PSUM bank, f32)

_NC_CACHE = {}


def _build_nc():
    import concourse.bacc as bacc
    import concourse.mybir as mybir
    import concourse.tile as tile

    f32 = mybir.dt.float32
    f32r = mybir.dt.float32r

    nc = bacc.Bacc()
    qT = nc.declare_dram_parameter("qT", [D, TOK], f32, isOutput=False)
    kT = nc.declare_dram_parameter("kT", [D, TOK], f32, isOutput=False)
    cosT = nc.declare_dram_parameter("cosT", [D, NSH], f32, isOutput=False)
    sinT = nc.declare_dram_parameter("sinT", [D, NSH], f32, isOutput=False)
    wA = nc.declare_dram_parameter("wA", [D, D], f32r, isOutput=False)
    wB = nc.declare_dram_parameter("wB", [D, D], f32r, isOutput=False)
    oqT = nc.declare_dram_parameter("oqT", [D, TOK], f32, isOutput=True)
    okT = nc.declare_dram_parameter("okT", [D, TOK], f32, isOutput=True)

    rep = CHUNK // NSH  # batches spanned by one chunk (chunks align to batches)
    assert CHUNK % NSH == 0 and TOK % CHUNK == 0 and CHUNK % MMN == 0

    with tile.TileContext(nc) as tc:
        with (
            tc.tile_pool(name="consts", bufs=1) as consts,
            tc.tile_pool(name="inp", bufs=3) as inp,
            tc.tile_pool(name="scaled", bufs=2) as sc,
            tc.tile_pool(name="outp", bufs=2) as outp,
            tc.tile_pool(name="pp", bufs=2, space="PSUM") as pp,
        ):
            a_t = consts.tile([D, D], f32r, tag="a", name="a_t")
            nc.sync.dma_start(out=a_t, in_=wA[:, :])
            b_t = consts.tile([D, D], f32r, tag="b", name="b_t")
            nc.sync.dma_start(out=b_t, in_=wB[:, :])
            cos_t = consts.tile([D, NSH], f32, tag="cos", name="cos_t")
            nc.sync.dma_start(out=cos_t, in_=cosT[:, :])
            sin_t = consts.tile([D, NSH], f32, tag="sin", name="sin_t")
            nc.sync.dma_start(out=sin_t, in_=sinT[:, :])

            cos_bc = cos_t.unsqueeze(1).broadcast_to((D, rep, NSH))
            sin_bc = sin_t.unsqueeze(1).broadcast_to((D, rep, NSH))

            # engine split tuned from the profile: DVE fp32 TT = 2282 ns/chunk-mul,
            # GpSimd = 4490 ns, PSUM->SBUF 1024-col copy ~1.2 us (ACT or DVE)
            def mul_engine(i, which):
                if which == "qs":
                    return nc.gpsimd
                if which == "ks" and i % 2 == 0:
                    return nc.gpsimd
                return nc.vector

            PS = 2 * MMN  # 1024-col psum tiles (2 banks); 2 tags x bufs=2 = 8 banks
            for i in range(TOK // CHUNK):
                csl = slice(i * CHUNK, (i + 1) * CHUNK)
                xq = inp.tile([D, CHUNK], f32, tag="xq", name="xq")
                nc.sync.dma_start(out=xq, in_=qT[:, csl])
                xk = inp.tile([D, CHUNK], f32, tag="xk", name="xk")
                nc.sync.dma_start(out=xk, in_=kT[:, csl])

                qc = sc.tile([D, CHUNK], f32r, tag="qc", name="qc")
                qs = sc.tile([D, CHUNK], f32r, tag="qs", name="qs")
                kc = sc.tile([D, CHUNK], f32r, tag="kc", name="kc")
                ks = sc.tile([D, CHUNK], f32r, tag="ks", name="ks")
                for x, xc_, xs_, cw, sw in (
                    (xq, qc, qs, "qc", "qs"),
                    (xk, kc, ks, "kc", "ks"),
                ):
                    x3 = x.rearrange("p (r n) -> p r n", n=NSH)
                    mul_engine(i, cw).tensor_mul(
                        xc_.rearrange("p (r n) -> p r n", n=NSH), x3, cos_bc
                    )
                    mul_engine(i, sw).tensor_mul(
                        xs_.rearrange("p (r n) -> p r n", n=NSH), x3, sin_bc
                    )

                oq = outp.tile([D, CHUNK], f32, tag="oq", name="oq")
                ok = outp.tile([D, CHUNK], f32, tag="ok", name="ok")
                for j2 in range(CHUNK // PS):
                    osl = slice(j2 * PS, (j2 + 1) * PS)
                    psq = pp.tile([D, PS], f32, tag="psq", name="psq")
                    psk = pp.tile([D, PS], f32, tag="psk", name="psk")
                    for h in range(PS // MMN):
                        sl = slice(j2 * PS + h * MMN, j2 * PS + (h + 1) * MMN)
                        psl = slice(h * MMN, (h + 1) * MMN)
                        nc.tensor.matmul(
                            psq[:, psl], a_t, qc[:, sl], start=True, stop=False
                        )
                        nc.tensor.matmul(
                            psk[:, psl], a_t, kc[:, sl], start=True, stop=False
                        )
                        nc.tensor.matmul(
                            psq[:, psl], b_t, qs[:, sl], start=False, stop=True
                        )
                        nc.tensor.matmul(
                            psk[:, psl], b_t, ks[:, sl], start=False, stop=True
                        )
                    nc.scalar.copy(out=oq[:, osl], in_=psq)
                    if i % 2 == 1 and j2 == 1:
                        nc.vector.tensor_copy(out=ok[:, osl], in_=psk)
                    else:
                        nc.scalar.copy(out=ok[:, osl], in_=psk)
                nc.sync.dma_start(out=oqT[:, csl], in_=oq)
                nc.sync.dma_start(out=okT[:, csl], in_=ok)

    nc.finalize()
    return nc


def _get_nc():
    if "nc" not in _NC_CACHE:
        _NC_CACHE["nc"] = _build_nc()
    return _NC_CACHE["nc"]


def _default_freqs():
    return (1.0 / 10000 ** (np.arange(0, D, 2, dtype=np.float64) / D)).astype(
        np.float32
    )


def _default_s_params():
    # Reproduce reference.setup_inputs()'s jax PRNG stream for s_params.
    import jax

    key = jax.random.key(0)
    _, _, k3 = jax.random.split(key, 3)
    num_s = D * (D - 1) // 2
    return np.asarray(
        0.02 * jax.random.normal(k3, (num_s,), dtype="float32"), dtype=np.float32
    )


def _host_prep(pos, freqs, s_params):
    """Cayley matrices (A, Bm as lhsT) and per-core cos/sin tables."""
    rows, cols = np.triu_indices(D, 1)
    S = np.zeros((D, D), np.float64)
    sp = np.asarray(s_params, dtype=np.float64)
    S[rows, cols] = sp
    S[cols, rows] = -sp
    I = np.eye(D)
    C = (I - S) @ np.linalg.inv(I + S)
    Bm = np.empty_like(C)
    Bm[:, 0::2] = C[:, 1::2]
    Bm[:, 1::2] = -C[:, 0::2]
    a_lhsT = np.ascontiguousarray(C.T.astype(np.float32))
    b_lhsT = np.ascontiguousarray(Bm.T.astype(np.float32))

    # angle computed in f32 to match the reference's rounding, trig in f64
    ang = np.asarray(freqs, np.float32)[:, None] * np.asarray(pos, np.float32)[None, :]
    ang64 = ang.astype(np.float64)
    cosT = np.repeat(np.cos(ang64), 2, axis=0).astype(np.float32)  # (D, N)
    sinT = np.repeat(np.sin(ang64), 2, axis=0).astype(np.float32)
    return a_lhsT, b_lhsT, cosT, sinT


LAST_RESULTS = None


def kernel(q, k, pos=None, freqs=None, s_params=None, _run_kwargs=None, **_ignored):
    q = np.ascontiguousarray(q, dtype=np.float32)
    k = np.ascontiguousarray(k, dtype=np.float32)
    if pos is None:
        pos = np.arange(N, dtype=np.float32)
    if freqs is None:
        freqs = _default_freqs()
    if s_params is None:
        s_params = _default_s_params()

    a_lhsT, b_lhsT, cosT, sinT = _host_prep(pos, freqs, s_params)

    in_maps = []
    for c in range(NCORES):
        ssl = slice(c * NSH, (c + 1) * NSH)
        qT = np.ascontiguousarray(q[:, ssl, :].reshape(TOK, D).T)
        kT = np.ascontiguousarray(k[:, ssl, :].reshape(TOK, D).T)
        in_maps.append(
            {
                "qT": qT,
                "kT": kT,
                "cosT": np.ascontiguousarray(cosT[:, ssl]),
                "sinT": np.ascontiguousarray(sinT[:, ssl]),
                "wA": a_lhsT,
                "wB": b_lhsT,
            }
        )

    from concourse.bass_utils import run_bass_kernel_spmd

    nc = _get_nc()
    res = run_bass_kernel_spmd(
        nc,
        in_maps,
        core_ids=list(range(NCORES)),
        **(_run_kwargs or {}),
    )
    global LAST_RESULTS
    LAST_RESULTS = res

    q_out = np.empty((B, N, D), np.float32)
    k_out = np.empty((B, N, D), np.float32)
    for c in range(NCORES):
        ssl = slice(c * NSH, (c + 1) * NSH)
        q_out[:, ssl, :] = res.results[c]["oqT"].T.reshape(B, NSH, D)
        k_out[:, ssl, :] = res.results[c]["okT"].T.reshape(B, NSH, D)
    return q_out, k_out


# revision 15
# speedup vs baseline: 1.0663x; 1.0663x over previous
"""Trainium2 Bass kernel for CayleyStringPE (RoPE + Cayley orthogonal mix).

Math: out = C @ rope(x) per token, where C = (I-S)(I+S)^{-1} is a fixed
128x128 orthogonal matrix (Cayley transform of the skew-symmetric S built
from s_params), and rope applies interleaved-pair rotations by angle
pos[t]*freqs[i].

Device formulation: rope(x)_t = x_t*c_t + P x_t * s_t with P the fixed
pair-swap-sign matrix and c_t/s_t the duplicated cos/sin vectors, so

    out_t = A @ (x_t * c_t) + Bm @ (x_t * s_t),   A = C,  Bm = C @ P

i.e. two 128x128 matmuls per token tile plus two elementwise multiplies.
No cross-partition shuffles on device.

Sharding: sequence-parallel across 8 cores (positions split 8 x 1024, all
batches on every core). cos/sin tables are per-core (128 x 1024) and reused
across the 8 batches. A/Bm replicated. No collectives.

Layout: tokens on the SBUF free axis, D=128 on partitions. Host pre-
transposes shards to (128, B*1024) D-major so all DMAs are contiguous.
"""

import sys

import numpy as np

for _p in ("/opt/trn_rl_repo", "/opt/pypackages"):
    if _p not in sys.path:
        sys.path.insert(0, _p)

B, N, D = 8, 8192, 128
NCORES = 8
NSH = N // NCORES          # positions per core
TOK = B * NSH              # tokens per core
CHUNK = 2048               # tokens per DMA chunk (1 MiB)
MMN = 512                  # matmul moving free dim (one PSUM bank, f32)

_NC_CACHE = {}


def _build_nc():
    import concourse.bacc as bacc
    import concourse.mybir as mybir
    import concourse.tile as tile

    f32 = mybir.dt.float32
    f32r = mybir.dt.float32r

    nc = bacc.Bacc()
    qT = nc.declare_dram_parameter("qT", [D, TOK], f32, isOutput=False)
    kT = nc.declare_dram_parameter("kT", [D, TOK], f32, isOutput=False)
    cosT = nc.declare_dram_parameter("cosT", [D, NSH], f32, isOutput=False)
    sinT = nc.declare_dram_parameter("sinT", [D, NSH], f32, isOutput=False)
    wA = nc.declare_dram_parameter("wA", [D, D], f32r, isOutput=False)
    wB = nc.declare_dram_parameter("wB", [D, D], f32r, isOutput=False)
    oqT = nc.declare_dram_parameter("oqT", [D, TOK], f32, isOutput=True)
    okT = nc.declare_dram_parameter("okT", [D, TOK], f32, isOutput=True)

    rep = CHUNK // NSH  # batches spanned by one chunk (chunks align to batches)
    assert CHUNK % NSH == 0 and TOK % CHUNK == 0 and CHUNK % MMN == 0

    with tile.TileContext(nc) as tc:
        with (
            tc.tile_pool(name="consts", bufs=1) as consts,
            tc.tile_pool(name="inp", bufs=3) as inp,
            tc.tile_pool(name="scaled", bufs=2) as sc,
            tc.tile_pool(name="outp", bufs=3) as outp,
            tc.tile_pool(name="pp", bufs=2, space="PSUM") as pp,
        ):
            a_t = consts.tile([D, D], f32r, tag="a", name="a_t")
            nc.sync.dma_start(out=a_t, in_=wA[:, :])
            b_t = consts.tile([D, D], f32r, tag="b", name="b_t")
            nc.sync.dma_start(out=b_t, in_=wB[:, :])
            cos_t = consts.tile([D, NSH], f32, tag="cos", name="cos_t")
            nc.sync.dma_start(out=cos_t, in_=cosT[:, :])
            sin_t = consts.tile([D, NSH], f32, tag="sin", name="sin_t")
            nc.sync.dma_start(out=sin_t, in_=sinT[:, :])

            cos_bc = cos_t.unsqueeze(1).broadcast_to((D, rep, NSH))
            sin_bc = sin_t.unsqueeze(1).broadcast_to((D, rep, NSH))

            # All muls on DVE (concurrent GpSimd+DVE SBUF traffic slows both),
            # all PSUM->SBUF copies on the otherwise-idle ACT engine.
            PS = 2 * MMN  # 1024-col psum tiles (2 banks); 2 tags x bufs=2 = 8 banks
            for i in range(TOK // CHUNK):
                csl = slice(i * CHUNK, (i + 1) * CHUNK)
                xq = inp.tile([D, CHUNK], f32, tag="xq", name="xq")
                nc.sync.dma_start(out=xq, in_=qT[:, csl])
                xk = inp.tile([D, CHUNK], f32, tag="xk", name="xk")
                nc.sync.dma_start(out=xk, in_=kT[:, csl])

                qc = sc.tile([D, CHUNK], f32r, tag="qc", name="qc")
                qs = sc.tile([D, CHUNK], f32r, tag="qs", name="qs")
                kc = sc.tile([D, CHUNK], f32r, tag="kc", name="kc")
                ks = sc.tile([D, CHUNK], f32r, tag="ks", name="ks")
                for x, xc_, xs_ in ((xq, qc, qs), (xk, kc, ks)):
                    x3 = x.rearrange("p (r n) -> p r n", n=NSH)
                    nc.vector.tensor_mul(
                        xc_.rearrange("p (r n) -> p r n", n=NSH), x3, cos_bc
                    )
                    nc.vector.tensor_mul(
                        xs_.rearrange("p (r n) -> p r n", n=NSH), x3, sin_bc
                    )

                oq = outp.tile([D, CHUNK], f32, tag="oq", name="oq")
                ok = outp.tile([D, CHUNK], f32, tag="ok", name="ok")
                for j2 in range(CHUNK // PS):
                    osl = slice(j2 * PS, (j2 + 1) * PS)
                    psq = pp.tile([D, PS], f32, tag="psq", name="psq")
                    psk = pp.tile([D, PS], f32, tag="psk", name="psk")
                    for h in range(PS // MMN):
                        sl = slice(j2 * PS + h * MMN, j2 * PS + (h + 1) * MMN)
                        psl = slice(h * MMN, (h + 1) * MMN)
                        nc.tensor.matmul(
                            psq[:, psl], a_t, qc[:, sl], start=True, stop=False
                        )
                        nc.tensor.matmul(
                            psk[:, psl], a_t, kc[:, sl], start=True, stop=False
                        )
                        nc.tensor.matmul(
                            psq[:, psl], b_t, qs[:, sl], start=False, stop=True
                        )
                        nc.tensor.matmul(
                            psk[:, psl], b_t, ks[:, sl], start=False, stop=True
                        )
                    nc.scalar.copy(out=oq[:, osl], in_=psq)
                    nc.scalar.copy(out=ok[:, osl], in_=psk)
                nc.sync.dma_start(out=oqT[:, csl], in_=oq)
                nc.sync.dma_start(out=okT[:, csl], in_=ok)

    nc.finalize()
    return nc


def _get_nc():
    if "nc" not in _NC_CACHE:
        _NC_CACHE["nc"] = _build_nc()
    return _NC_CACHE["nc"]


def _default_freqs():
    return (1.0 / 10000 ** (np.arange(0, D, 2, dtype=np.float64) / D)).astype(
        np.float32
    )


def _default_s_params():
    # Reproduce reference.setup_inputs()'s jax PRNG stream for s_params.
    import jax

    key = jax.random.key(0)
    _, _, k3 = jax.random.split(key, 3)
    num_s = D * (D - 1) // 2
    return np.asarray(
        0.02 * jax.random.normal(k3, (num_s,), dtype="float32"), dtype=np.float32
    )


def _host_prep(pos, freqs, s_params):
    """Cayley matrices (A, Bm as lhsT) and per-core cos/sin tables."""
    rows, cols = np.triu_indices(D, 1)
    S = np.zeros((D, D), np.float64)
    sp = np.asarray(s_params, dtype=np.float64)
    S[rows, cols] = sp
    S[cols, rows] = -sp
    I = np.eye(D)
    C = (I - S) @ np.linalg.inv(I + S)
    Bm = np.empty_like(C)
    Bm[:, 0::2] = C[:, 1::2]
    Bm[:, 1::2] = -C[:, 0::2]
    a_lhsT = np.ascontiguousarray(C.T.astype(np.float32))
    b_lhsT = np.ascontiguousarray(Bm.T.astype(np.float32))

    # angle computed in f32 to match the reference's rounding, trig in f64
    ang = np.asarray(freqs, np.float32)[:, None] * np.asarray(pos, np.float32)[None, :]
    ang64 = ang.astype(np.float64)
    cosT = np.repeat(np.cos(ang64), 2, axis=0).astype(np.float32)  # (D, N)
    sinT = np.repeat(np.sin(ang64), 2, axis=0).astype(np.float32)
    return a_lhsT, b_lhsT, cosT, sinT


LAST_RESULTS = None


def kernel(q, k, pos=None, freqs=None, s_params=None, _run_kwargs=None, **_ignored):
    q = np.ascontiguousarray(q, dtype=np.float32)
    k = np.ascontiguousarray(k, dtype=np.float32)
    if pos is None:
        pos = np.arange(N, dtype=np.float32)
    if freqs is None:
        freqs = _default_freqs()
    if s_params is None:
        s_params = _default_s_params()

    a_lhsT, b_lhsT, cosT, sinT = _host_prep(pos, freqs, s_params)

    in_maps = []
    for c in range(NCORES):
        ssl = slice(c * NSH, (c + 1) * NSH)
        qT = np.ascontiguousarray(q[:, ssl, :].reshape(TOK, D).T)
        kT = np.ascontiguousarray(k[:, ssl, :].reshape(TOK, D).T)
        in_maps.append(
            {
                "qT": qT,
                "kT": kT,
                "cosT": np.ascontiguousarray(cosT[:, ssl]),
                "sinT": np.ascontiguousarray(sinT[:, ssl]),
                "wA": a_lhsT,
                "wB": b_lhsT,
            }
        )

    from concourse.bass_utils import run_bass_kernel_spmd

    nc = _get_nc()
    res = run_bass_kernel_spmd(
        nc,
        in_maps,
        core_ids=list(range(NCORES)),
        **(_run_kwargs or {}),
    )
    global LAST_RESULTS
    LAST_RESULTS = res

    q_out = np.empty((B, N, D), np.float32)
    k_out = np.empty((B, N, D), np.float32)
    for c in range(NCORES):
        ssl = slice(c * NSH, (c + 1) * NSH)
        q_out[:, ssl, :] = res.results[c]["oqT"].T.reshape(B, NSH, D)
        k_out[:, ssl, :] = res.results[c]["okT"].T.reshape(B, NSH, D)
    return q_out, k_out


# revision 16
# speedup vs baseline: 1.6058x; 1.5059x over previous
"""Trainium2 Bass kernel for CayleyStringPE (RoPE + Cayley orthogonal mix).

Math: out = C @ rope(x) per token, where C = (I-S)(I+S)^{-1} is a fixed
128x128 orthogonal matrix (Cayley transform of the skew-symmetric S built
from s_params), and rope applies interleaved-pair rotations by angle
pos[t]*freqs[i].

Device formulation: rope(x)_t = x_t*c_t + P x_t * s_t with P the fixed
pair-swap-sign matrix and c_t/s_t the duplicated cos/sin vectors, so

    out_t = A @ (x_t * c_t) + Bm @ (x_t * s_t),   A = C,  Bm = C @ P

i.e. two 128x128 matmuls per token tile plus two elementwise multiplies.
No cross-partition shuffles on device.

Precision: fp16 end-to-end (inputs, trig tables, weights, outputs) with
f32 PSUM accumulation. fp16's 11-bit mantissa keeps the overall relative
error at ~3e-4 while unlocking the DVE 2x tensor-tensor mode, FWL fast
weight loads, and half the HBM traffic of f32.

Sharding: sequence-parallel across 8 cores (positions split 8 x 1024, all
batches on every core). cos/sin tables are per-core (128 x 1024) and reused
across the 8 batches. A/Bm replicated. No collectives.

Layout: tokens on the SBUF free axis, D=128 on partitions. Host pre-
transposes shards to (128, B*1024) D-major so all DMAs are contiguous.
"""

import sys

import numpy as np

for _p in ("/opt/trn_rl_repo", "/opt/pypackages"):
    if _p not in sys.path:
        sys.path.insert(0, _p)

B, N, D = 8, 8192, 128
NCORES = 8
NSH = N // NCORES          # positions per core
TOK = B * NSH              # tokens per core
CHUNK = 4096               # tokens per DMA chunk (1 MiB at fp16)
MMN = 512                  # matmul moving free dim (one PSUM bank, f32)

_NC_CACHE = {}


def _build_nc():
    import concourse.bacc as bacc
    import concourse.mybir as mybir
    import concourse.tile as tile

    f16 = mybir.dt.float16
    f32 = mybir.dt.float32

    nc = bacc.Bacc()
    qT = nc.declare_dram_parameter("qT", [D, TOK], f16, isOutput=False)
    kT = nc.declare_dram_parameter("kT", [D, TOK], f16, isOutput=False)
    cosT = nc.declare_dram_parameter("cosT", [D, NSH], f16, isOutput=False)
    sinT = nc.declare_dram_parameter("sinT", [D, NSH], f16, isOutput=False)
    wA = nc.declare_dram_parameter("wA", [D, D], f16, isOutput=False)
    wB = nc.declare_dram_parameter("wB", [D, D], f16, isOutput=False)
    oqT = nc.declare_dram_parameter("oqT", [D, TOK], f16, isOutput=True)
    okT = nc.declare_dram_parameter("okT", [D, TOK], f16, isOutput=True)

    rep = CHUNK // NSH  # batches spanned by one chunk (chunks align to batches)
    assert CHUNK % NSH == 0 and TOK % CHUNK == 0 and CHUNK % MMN == 0

    with tile.TileContext(nc) as tc:
        with (
            tc.tile_pool(name="consts", bufs=1) as consts,
            tc.tile_pool(name="inp", bufs=3) as inp,
            tc.tile_pool(name="scaled", bufs=2) as sc,
            tc.tile_pool(name="outp", bufs=3) as outp,
            tc.tile_pool(name="pp", bufs=2, space="PSUM") as pp,
        ):
            a_t = consts.tile([D, D], f16, tag="a", name="a_t")
            nc.sync.dma_start(out=a_t, in_=wA[:, :])
            b_t = consts.tile([D, D], f16, tag="b", name="b_t")
            nc.sync.dma_start(out=b_t, in_=wB[:, :])
            cos_t = consts.tile([D, NSH], f16, tag="cos", name="cos_t")
            nc.sync.dma_start(out=cos_t, in_=cosT[:, :])
            sin_t = consts.tile([D, NSH], f16, tag="sin", name="sin_t")
            nc.sync.dma_start(out=sin_t, in_=sinT[:, :])

            cos_bc = cos_t.unsqueeze(1).broadcast_to((D, rep, NSH))
            sin_bc = sin_t.unsqueeze(1).broadcast_to((D, rep, NSH))

            PS = 2 * MMN  # 1024-col psum tiles (2 banks); 2 tags x bufs=2 = 8 banks
            for i in range(TOK // CHUNK):
                csl = slice(i * CHUNK, (i + 1) * CHUNK)
                xq = inp.tile([D, CHUNK], f16, tag="xq", name="xq")
                nc.sync.dma_start(out=xq, in_=qT[:, csl])
                xk = inp.tile([D, CHUNK], f16, tag="xk", name="xk")
                nc.sync.dma_start(out=xk, in_=kT[:, csl])

                qc = sc.tile([D, CHUNK], f16, tag="qc", name="qc")
                qs = sc.tile([D, CHUNK], f16, tag="qs", name="qs")
                kc = sc.tile([D, CHUNK], f16, tag="kc", name="kc")
                ks = sc.tile([D, CHUNK], f16, tag="ks", name="ks")
                for x, xc_, xs_ in ((xq, qc, qs), (xk, kc, ks)):
                    x3 = x.rearrange("p (r n) -> p r n", n=NSH)
                    nc.vector.tensor_mul(
                        xc_.rearrange("p (r n) -> p r n", n=NSH), x3, cos_bc
                    )
                    nc.vector.tensor_mul(
                        xs_.rearrange("p (r n) -> p r n", n=NSH), x3, sin_bc
                    )

                oq = outp.tile([D, CHUNK], f16, tag="oq", name="oq")
                ok = outp.tile([D, CHUNK], f16, tag="ok", name="ok")
                for j2 in range(CHUNK // PS):
                    osl = slice(j2 * PS, (j2 + 1) * PS)
                    psq = pp.tile([D, PS], f32, tag="psq", name="psq")
                    psk = pp.tile([D, PS], f32, tag="psk", name="psk")
                    for h in range(PS // MMN):
                        sl = slice(j2 * PS + h * MMN, j2 * PS + (h + 1) * MMN)
                        psl = slice(h * MMN, (h + 1) * MMN)
                        nc.tensor.matmul(
                            psq[:, psl], a_t, qc[:, sl], start=True, stop=False
                        )
                        nc.tensor.matmul(
                            psk[:, psl], a_t, kc[:, sl], start=True, stop=False
                        )
                        nc.tensor.matmul(
                            psq[:, psl], b_t, qs[:, sl], start=False, stop=True
                        )
                        nc.tensor.matmul(
                            psk[:, psl], b_t, ks[:, sl], start=False, stop=True
                        )
                    nc.scalar.copy(out=oq[:, osl], in_=psq)
                    nc.vector.tensor_copy(out=ok[:, osl], in_=psk)
                nc.sync.dma_start(out=oqT[:, csl], in_=oq)
                nc.sync.dma_start(out=okT[:, csl], in_=ok)

    nc.finalize()
    return nc


def _get_nc():
    if "nc" not in _NC_CACHE:
        _NC_CACHE["nc"] = _build_nc()
    return _NC_CACHE["nc"]


def _default_freqs():
    return (1.0 / 10000 ** (np.arange(0, D, 2, dtype=np.float64) / D)).astype(
        np.float32
    )


def _default_s_params():
    # Reproduce reference.setup_inputs()'s jax PRNG stream for s_params.
    import jax

    key = jax.random.key(0)
    _, _, k3 = jax.random.split(key, 3)
    num_s = D * (D - 1) // 2
    return np.asarray(
        0.02 * jax.random.normal(k3, (num_s,), dtype="float32"), dtype=np.float32
    )


def _host_prep(pos, freqs, s_params):
    """Cayley matrices (A, Bm as lhsT) and cos/sin tables, all fp16."""
    rows, cols = np.triu_indices(D, 1)
    S = np.zeros((D, D), np.float64)
    sp = np.asarray(s_params, dtype=np.float64)
    S[rows, cols] = sp
    S[cols, rows] = -sp
    I = np.eye(D)
    C = (I - S) @ np.linalg.inv(I + S)
    Bm = np.empty_like(C)
    Bm[:, 0::2] = C[:, 1::2]
    Bm[:, 1::2] = -C[:, 0::2]
    a_lhsT = np.ascontiguousarray(C.T.astype(np.float16))
    b_lhsT = np.ascontiguousarray(Bm.T.astype(np.float16))

    # angle computed in f32 to match the reference's rounding, trig in f64
    ang = np.asarray(freqs, np.float32)[:, None] * np.asarray(pos, np.float32)[None, :]
    ang64 = ang.astype(np.float64)
    cosT = np.repeat(np.cos(ang64), 2, axis=0).astype(np.float16)  # (D, N)
    sinT = np.repeat(np.sin(ang64), 2, axis=0).astype(np.float16)
    return a_lhsT, b_lhsT, cosT, sinT


LAST_RESULTS = None


def kernel(q, k, pos=None, freqs=None, s_params=None, _run_kwargs=None, **_ignored):
    q = np.asarray(q, dtype=np.float32)
    k = np.asarray(k, dtype=np.float32)
    if pos is None:
        pos = np.arange(N, dtype=np.float32)
    if freqs is None:
        freqs = _default_freqs()
    if s_params is None:
        s_params = _default_s_params()

    a_lhsT, b_lhsT, cosT, sinT = _host_prep(pos, freqs, s_params)

    q16 = q.astype(np.float16)
    k16 = k.astype(np.float16)

    in_maps = []
    for c in range(NCORES):
        ssl = slice(c * NSH, (c + 1) * NSH)
        qT = np.ascontiguousarray(q16[:, ssl, :].reshape(TOK, D).T)
        kT = np.ascontiguousarray(k16[:, ssl, :].reshape(TOK, D).T)
        in_maps.append(
            {
                "qT": qT,
                "kT": kT,
                "cosT": np.ascontiguousarray(cosT[:, ssl]),
                "sinT": np.ascontiguousarray(sinT[:, ssl]),
                "wA": a_lhsT,
                "wB": b_lhsT,
            }
        )

    from concourse.bass_utils import run_bass_kernel_spmd

    nc = _get_nc()
    res = run_bass_kernel_spmd(
        nc,
        in_maps,
        core_ids=list(range(NCORES)),
        **(_run_kwargs or {}),
    )
    global LAST_RESULTS
    LAST_RESULTS = res

    q_out = np.empty((B, N, D), np.float32)
    k_out = np.empty((B, N, D), np.float32)
    for c in range(NCORES):
        ssl = slice(c * NSH, (c + 1) * NSH)
        q_out[:, ssl, :] = res.results[c]["oqT"].T.reshape(B, NSH, D).astype(np.float32)
        k_out[:, ssl, :] = res.results[c]["okT"].T.reshape(B, NSH, D).astype(np.float32)
    return q_out, k_out


# revision 17
# speedup vs baseline: 1.7909x; 1.1153x over previous
"""Trainium2 Bass kernel for CayleyStringPE (RoPE + Cayley orthogonal mix).

Math: out = C @ rope(x) per token, where C = (I-S)(I+S)^{-1} is a fixed
128x128 orthogonal matrix (Cayley transform of the skew-symmetric S built
from s_params), and rope applies interleaved-pair rotations by angle
pos[t]*freqs[i].

Device formulation: rope(x)_t = x_t*c_t + P x_t * s_t with P the fixed
pair-swap-sign matrix and c_t/s_t the duplicated cos/sin vectors, so

    out_t = A @ (x_t * c_t) + Bm @ (x_t * s_t),   A = C,  Bm = C @ P

i.e. two 128x128 matmuls per token tile plus two elementwise multiplies.
No cross-partition shuffles on device.

Precision: fp16 end-to-end (inputs, trig tables, weights, outputs) with
f32 PSUM accumulation. fp16's 11-bit mantissa keeps the overall relative
error at ~3e-4 while unlocking the DVE 2x tensor-tensor mode, FWL fast
weight loads, and half the HBM traffic of f32.

Sharding: sequence-parallel across 8 cores (positions split 8 x 1024, all
batches on every core). cos/sin tables are per-core (128 x 1024) and reused
across the 8 batches. A/Bm replicated. No collectives.

Layout: tokens on the SBUF free axis, D=128 on partitions. Host pre-
transposes shards to (128, B*1024) D-major so all DMAs are contiguous.
"""

import sys

import numpy as np

for _p in ("/opt/trn_rl_repo", "/opt/pypackages"):
    if _p not in sys.path:
        sys.path.insert(0, _p)

B, N, D = 8, 8192, 128
NCORES = 8
NSH = N // NCORES          # positions per core
TOK = B * NSH              # tokens per core
CHUNK = 2048               # tokens per DMA chunk (0.5 MiB at fp16)
MMN = 512                  # matmul moving free dim (one PSUM bank, f32)

_NC_CACHE = {}


def _build_nc():
    import concourse.bacc as bacc
    import concourse.mybir as mybir
    import concourse.tile as tile

    f16 = mybir.dt.float16
    f32 = mybir.dt.float32

    nc = bacc.Bacc()
    qT = nc.declare_dram_parameter("qT", [D, TOK], f16, isOutput=False)
    kT = nc.declare_dram_parameter("kT", [D, TOK], f16, isOutput=False)
    cosT = nc.declare_dram_parameter("cosT", [D, NSH], f16, isOutput=False)
    sinT = nc.declare_dram_parameter("sinT", [D, NSH], f16, isOutput=False)
    wA = nc.declare_dram_parameter("wA", [D, D], f16, isOutput=False)
    wB = nc.declare_dram_parameter("wB", [D, D], f16, isOutput=False)
    oqT = nc.declare_dram_parameter("oqT", [D, TOK], f16, isOutput=True)
    okT = nc.declare_dram_parameter("okT", [D, TOK], f16, isOutput=True)

    rep = CHUNK // NSH  # batches spanned by one chunk (chunks align to batches)
    assert CHUNK % NSH == 0 and TOK % CHUNK == 0 and CHUNK % MMN == 0

    with tile.TileContext(nc) as tc:
        with (
            tc.tile_pool(name="consts", bufs=1) as consts,
            tc.tile_pool(name="inp", bufs=3) as inp,
            tc.tile_pool(name="scaled", bufs=2) as sc,
            tc.tile_pool(name="outp", bufs=3) as outp,
            tc.tile_pool(name="pp", bufs=2, space="PSUM") as pp,
        ):
            a_t = consts.tile([D, D], f16, tag="a", name="a_t")
            nc.sync.dma_start(out=a_t, in_=wA[:, :])
            b_t = consts.tile([D, D], f16, tag="b", name="b_t")
            nc.sync.dma_start(out=b_t, in_=wB[:, :])
            cos_t = consts.tile([D, NSH], f16, tag="cos", name="cos_t")
            nc.sync.dma_start(out=cos_t, in_=cosT[:, :])
            sin_t = consts.tile([D, NSH], f16, tag="sin", name="sin_t")
            nc.sync.dma_start(out=sin_t, in_=sinT[:, :])

            cos_bc = cos_t.unsqueeze(1).broadcast_to((D, rep, NSH))
            sin_bc = sin_t.unsqueeze(1).broadcast_to((D, rep, NSH))

            PS = 2 * MMN  # 1024-col psum tiles (2 banks); 2 tags x bufs=2 = 8 banks
            for i in range(TOK // CHUNK):
                csl = slice(i * CHUNK, (i + 1) * CHUNK)
                xq = inp.tile([D, CHUNK], f16, tag="xq", name="xq")
                nc.sync.dma_start(out=xq, in_=qT[:, csl])
                xk = inp.tile([D, CHUNK], f16, tag="xk", name="xk")
                nc.sync.dma_start(out=xk, in_=kT[:, csl])

                qc = sc.tile([D, CHUNK], f16, tag="qc", name="qc")
                qs = sc.tile([D, CHUNK], f16, tag="qs", name="qs")
                kc = sc.tile([D, CHUNK], f16, tag="kc", name="kc")
                ks = sc.tile([D, CHUNK], f16, tag="ks", name="ks")
                for x, xc_, xs_ in ((xq, qc, qs), (xk, kc, ks)):
                    x3 = x.rearrange("p (r n) -> p r n", n=NSH)
                    nc.vector.tensor_mul(
                        xc_.rearrange("p (r n) -> p r n", n=NSH), x3, cos_bc
                    )
                    nc.vector.tensor_mul(
                        xs_.rearrange("p (r n) -> p r n", n=NSH), x3, sin_bc
                    )

                oq = outp.tile([D, CHUNK], f16, tag="oq", name="oq")
                ok = outp.tile([D, CHUNK], f16, tag="ok", name="ok")
                for j2 in range(CHUNK // PS):
                    osl = slice(j2 * PS, (j2 + 1) * PS)
                    psq = pp.tile([D, PS], f32, tag="psq", name="psq")
                    psk = pp.tile([D, PS], f32, tag="psk", name="psk")
                    for h in range(PS // MMN):
                        sl = slice(j2 * PS + h * MMN, j2 * PS + (h + 1) * MMN)
                        psl = slice(h * MMN, (h + 1) * MMN)
                        nc.tensor.matmul(
                            psq[:, psl], a_t, qc[:, sl], start=True, stop=False
                        )
                        nc.tensor.matmul(
                            psk[:, psl], a_t, kc[:, sl], start=True, stop=False
                        )
                        nc.tensor.matmul(
                            psq[:, psl], b_t, qs[:, sl], start=False, stop=True
                        )
                        nc.tensor.matmul(
                            psk[:, psl], b_t, ks[:, sl], start=False, stop=True
                        )
                    nc.scalar.copy(out=oq[:, osl], in_=psq)
                    nc.scalar.copy(out=ok[:, osl], in_=psk)
                nc.sync.dma_start(out=oqT[:, csl], in_=oq)
                nc.sync.dma_start(out=okT[:, csl], in_=ok)

    nc.finalize()
    return nc


def _get_nc():
    if "nc" not in _NC_CACHE:
        _NC_CACHE["nc"] = _build_nc()
    return _NC_CACHE["nc"]


def _default_freqs():
    return (1.0 / 10000 ** (np.arange(0, D, 2, dtype=np.float64) / D)).astype(
        np.float32
    )


def _default_s_params():
    # Reproduce reference.setup_inputs()'s jax PRNG stream for s_params.
    import jax

    key = jax.random.key(0)
    _, _, k3 = jax.random.split(key, 3)
    num_s = D * (D - 1) // 2
    return np.asarray(
        0.02 * jax.random.normal(k3, (num_s,), dtype="float32"), dtype=np.float32
    )


def _host_prep(pos, freqs, s_params):
    """Cayley matrices (A, Bm as lhsT) and cos/sin tables, all fp16."""
    rows, cols = np.triu_indices(D, 1)
    S = np.zeros((D, D), np.float64)
    sp = np.asarray(s_params, dtype=np.float64)
    S[rows, cols] = sp
    S[cols, rows] = -sp
    I = np.eye(D)
    C = (I - S) @ np.linalg.inv(I + S)
    Bm = np.empty_like(C)
    Bm[:, 0::2] = C[:, 1::2]
    Bm[:, 1::2] = -C[:, 0::2]
    a_lhsT = np.ascontiguousarray(C.T.astype(np.float16))
    b_lhsT = np.ascontiguousarray(Bm.T.astype(np.float16))

    # angle computed in f32 to match the reference's rounding, trig in f64
    ang = np.asarray(freqs, np.float32)[:, None] * np.asarray(pos, np.float32)[None, :]
    ang64 = ang.astype(np.float64)
    cosT = np.repeat(np.cos(ang64), 2, axis=0).astype(np.float16)  # (D, N)
    sinT = np.repeat(np.sin(ang64), 2, axis=0).astype(np.float16)
    return a_lhsT, b_lhsT, cosT, sinT


LAST_RESULTS = None


def kernel(q, k, pos=None, freqs=None, s_params=None, _run_kwargs=None, **_ignored):
    q = np.asarray(q, dtype=np.float32)
    k = np.asarray(k, dtype=np.float32)
    if pos is None:
        pos = np.arange(N, dtype=np.float32)
    if freqs is None:
        freqs = _default_freqs()
    if s_params is None:
        s_params = _default_s_params()

    a_lhsT, b_lhsT, cosT, sinT = _host_prep(pos, freqs, s_params)

    q16 = q.astype(np.float16)
    k16 = k.astype(np.float16)

    in_maps = []
    for c in range(NCORES):
        ssl = slice(c * NSH, (c + 1) * NSH)
        qT = np.ascontiguousarray(q16[:, ssl, :].reshape(TOK, D).T)
        kT = np.ascontiguousarray(k16[:, ssl, :].reshape(TOK, D).T)
        in_maps.append(
            {
                "qT": qT,
                "kT": kT,
                "cosT": np.ascontiguousarray(cosT[:, ssl]),
                "sinT": np.ascontiguousarray(sinT[:, ssl]),
                "wA": a_lhsT,
                "wB": b_lhsT,
            }
        )

    from concourse.bass_utils import run_bass_kernel_spmd

    nc = _get_nc()
    res = run_bass_kernel_spmd(
        nc,
        in_maps,
        core_ids=list(range(NCORES)),
        **(_run_kwargs or {}),
    )
    global LAST_RESULTS
    LAST_RESULTS = res

    q_out = np.empty((B, N, D), np.float32)
    k_out = np.empty((B, N, D), np.float32)
    for c in range(NCORES):
        ssl = slice(c * NSH, (c + 1) * NSH)
        q_out[:, ssl, :] = res.results[c]["oqT"].T.reshape(B, NSH, D).astype(np.float32)
        k_out[:, ssl, :] = res.results[c]["okT"].T.reshape(B, NSH, D).astype(np.float32)
    return q_out, k_out
